# revision 20
# baseline (speedup 1.0000x reference)
"""ALiBi attention (B=2, S=2048, D=1024, H=16) on 8 TRN2 NeuronCores.

Sharding: core c handles batch b = c//4 and query slice qs = (c%4)*512.
Key insight: the reference applies bias slope_h*(k-q) with NO causal mask and
slopes in [0.52, 1.0], so softmax mass sits entirely on the last ~60 keys.
Keeping only the last KW=128 keys gives max attention-weight error ~1e-29.
Furthermore exp(qk*scale + slope*(k-q) - rowmax(q)) with rowmax ~= slope*(S-1-q)
reduces to exp(qk*scale + slope*(k-S+1)): the bias is purely a function of k,
i.e. a per-partition constant in the [k, q] layout -> single fused ACT op.

Per core:
  QT[ch,q]   = Wq^T x^T        (8 ch-tiles x 8 d-tiles, N=512)
  KT[ch,k]   = Wk^T xk^T       (8 x 8, N=128)
  V[k,ch]    = xk Wv           (2 blocks x 8 d-tiles, N=512) + ones col per head
  PT_h[k,q]  = exp(scale*KT_h^T QT_h + cbias_h)      (ACT, per-partition bias)
  denom      = ones^T PT_h     -> reciprocal -> PE outer-product broadcast
  outT_h     = V_h^T PT_h, normalized by denom       (col-tiled into head pairs)
  out[q,d]   = outT^T Wo       (+ bo on host)
No collectives: cores are fully independent; host concatenates query slices.
"""

import numpy as np
import ml_dtypes

D = 1024
H = 16
HD = 64
B = 2
S = 2048
QS = 512          # queries per core
KW = 128          # key window (last KW keys carry all softmax mass)
K0 = S - KW
NT = 8            # 128-wide tiles over D
P = 128
SCALE = HD ** -0.5
N_CORES = 8

_CACHE = {}


def _build():
    import concourse.bacc as bacc
    import concourse.mybir as mybir
    import concourse.tile as tile
    from concourse.masks import make_identity

    BF = mybir.dt.bfloat16
    F32 = mybir.dt.float32
    AF = mybir.ActivationFunctionType

    nc = bacc.Bacc("TRN2", target_bir_lowering=False, debug=False, num_devices=N_CORES)

    xTq = nc.dram_tensor("xTq", [D, QS], BF, kind="ExternalInput").ap()
    xTk = nc.dram_tensor("xTk", [D, KW], BF, kind="ExternalInput").ap()
    Wq = nc.dram_tensor("Wq", [D, D], BF, kind="ExternalInput").ap()
    Wk = nc.dram_tensor("Wk", [D, D], BF, kind="ExternalInput").ap()
    Wv = nc.dram_tensor("Wv", [D, D], BF, kind="ExternalInput").ap()
    Wo = nc.dram_tensor("Wo", [D, D], BF, kind="ExternalInput").ap()
    bq = nc.dram_tensor("bq", [P, NT], F32, kind="ExternalInput").ap()
    bk = nc.dram_tensor("bk", [P, NT], F32, kind="ExternalInput").ap()
    bv = nc.dram_tensor("bv", [1, D], BF, kind="ExternalInput").ap()
    cb = nc.dram_tensor("cbias", [KW, H], F32, kind="ExternalInput").ap()
    out = nc.dram_tensor("out", [QS, D], F32, kind="ExternalOutput").ap()

    with tile.TileContext(nc) as tc:
        with (
            tc.tile_pool(name="wpool", bufs=1) as wp,
            tc.tile_pool(name="dpool", bufs=1) as dp,
            tc.tile_pool(name="flow", bufs=3) as fp,
            tc.tile_pool(name="pacc", bufs=2, space="PSUM") as pacc,
            tc.tile_pool(name="patt", bufs=2, space="PSUM") as patt,
            tc.tile_pool(name="psml", bufs=1, space="PSUM") as psml,
        ):
            # ---- load inputs. Per-d-tile chunks (256KB) alternating across the
            # two HWDGE rings (sync + scalar) so accumulation loops can start on
            # chunk 0 instead of waiting for a whole 2MB tensor. Tensor order =
            # compute need order: xk, wk (K), xq, wq (QT), wv (V), wo (outproj).
            rings = [nc.sync, nc.scalar]
            xk_a = dp.tile([P, NT, KW], BF, tag="xk_a")
            nc.sync.dma_start(xk_a[:], xTk.rearrange("(t p) k -> p t k", p=P))

            def load_chunked(name, pool, src, cols, first=0):
                a = pool.tile([P, NT, cols], BF, tag=name, name=name)
                src3 = src.rearrange("(t p) c -> p t c", p=P)
                for t in range(first, NT):
                    rings[t % 2].dma_start(a[:, t], src3[:, t])
                return a

            wk_a = wp.tile([P, NT, D], BF, tag="wk_a")
            wk_src = Wk.rearrange("(t p) c -> p t c", p=P)
            for t in range(2):
                rings[(t + 1) % 2].dma_start(wk_a[:, t], wk_src[:, t])
            # small tensors next: needed early-ish but must not delay wk chunk 0
            bq_a = dp.tile([P, NT], F32, tag="bq_a")
            nc.sync.dma_start(bq_a[:], bq[:])
            bk_a = dp.tile([P, NT], F32, tag="bk_a")
            nc.scalar.dma_start(bk_a[:], bk[:])
            cb_a = dp.tile([KW, H], F32, tag="cb_a")
            nc.sync.dma_start(cb_a[:], cb[:])
            bv_sb = dp.tile([1, D], BF, tag="bvsb")
            nc.scalar.dma_start(bv_sb[:], bv[:])
            bq_t = [bq_a[:, t:t + 1] for t in range(NT)]
            bk_t = [bk_a[:, t:t + 1] for t in range(NT)]
            cb_t = [cb_a[:, h:h + 1] for h in range(H)]
            for t in range(2, NT):
                rings[(t + 1) % 2].dma_start(wk_a[:, t], wk_src[:, t])

            xq_a = load_chunked("xq_a", dp, xTq, QS)
            wq_a = load_chunked("wq_a", wp, Wq, D)
            wv_a = load_chunked("wv_a", wp, Wv, D)
            wo_a = load_chunked("wo_a", wp, Wo, D)
            xk_t = [xk_a[:, t] for t in range(NT)]
            wk_t = [wk_a[:, t] for t in range(NT)]
            wv_t = [wv_a[:, t] for t in range(NT)]
            xq_t = [xq_a[:, t] for t in range(NT)]
            wq_t = [wq_a[:, t] for t in range(NT)]
            wo_t = [wo_a[:, t] for t in range(NT)]

            ones_row = dp.tile([1, P], BF, tag="ones_row")
            nc.vector.memset(ones_row[:], 1.0)
            # one-hot stationaries: oh[:, h*16 + (h%8)] = 1 -> den-stack matmuls
            oh = dp.tile([P, H * 16], BF, tag="onehot")
            nc.vector.memset(oh[:], 0.0)
            for h in range(H):
                nc.vector.memset(oh[:, h * 16 + (h % 8):h * 16 + (h % 8) + 1], 1.0)

            identity = dp.tile([P, P], BF, tag="identity")
            make_identity(nc, identity[:])

            # ---- K[k, ch] = xk^T Wk (16 big matmuls, d-outer, paced by wk
            # chunks), then 8 PE transposes -> KT[ch, k] + bias ----
            k_sb = dp.tile([P, D], BF, tag="ksb")
            kps = [
                pacc.tile([P, 512], F32, tag="acc", name=f"kps{_b}")
                for _b in range(2)
            ]
            for d in range(NT):
                for blk in range(2):
                    nc.tensor.matmul(
                        kps[blk][:], xk_t[d][:],
                        wk_t[d][:, blk * 512:(blk + 1) * 512],
                        start=(d == 0), stop=(d == NT - 1),
                    )
            for blk in range(2):
                nc.vector.tensor_copy(k_sb[:, blk * 512:(blk + 1) * 512], kps[blk][:])
            kt_t = []
            for t in range(NT):
                tps = patt.tile([P, P], BF, tag="scores", name=f"tps{t}")
                nc.tensor.transpose(tps[:], k_sb[:, t * P:(t + 1) * P], identity[:])
                kt = dp.tile([P, KW], BF, tag=f"kt{t}", name=f"kt{t}")
                nc.vector.tensor_scalar_add(kt[:], tps[:], bk_t[t][:])
                kt_t.append(kt)

            # ---- QT[ch, q]: d-outer over ch-tile pairs, paced by wq chunk DMAs ----
            qt_t = []
            for pr in range(NT // 2):
                ps2 = [
                    pacc.tile([P, QS], F32, tag="acc", name=f"qps{pr}_{j}")
                    for j in range(2)
                ]
                for d in range(NT):
                    for j in range(2):
                        t = 2 * pr + j
                        nc.tensor.matmul(
                            ps2[j][:], wq_t[d][:, t * P:(t + 1) * P], xq_t[d][:],
                            start=(d == 0), stop=(d == NT - 1),
                        )
                for j in range(2):
                    t = 2 * pr + j
                    qt = dp.tile([P, QS], BF, tag=f"qt{t}", name=f"qt{t}")
                    nc.vector.tensor_scalar_add(qt[:], ps2[j][:], bq_t[t][:])
                    qt_t.append(qt)

            # ---- V[k, ch] (stationary xk_t[d] reused across both 512-blocks) ----
            v_sb = dp.tile([P, D], BF, tag="vsb")
            vps = [pacc.tile([P, 512], F32, tag="acc", name=f"vps{_b}") for _b in range(2)]
            for d in range(NT):
                for blk in range(2):
                    nc.tensor.matmul(
                        vps[blk][:], xk_t[d][:],
                        wv_t[d][:, blk * 512:(blk + 1) * 512],
                        start=(d == 0), stop=False,
                    )
            for blk in range(2):
                nc.tensor.matmul(
                    vps[blk][:], ones_row[:], bv_sb[:, blk * 512:(blk + 1) * 512],
                    start=False, stop=True,
                )
                nc.vector.tensor_copy(v_sb[:, blk * 512:(blk + 1) * 512], vps[blk][:])

            # ---- attention ----
            # Phase A per head: QK -> exp(PT) -> one-hot den-stack matmul; PV pairs.
            # Denominators for heads 0-7 accumulate in den_ps[0], 8-15 in den_ps[1]
            # (rows h%8). One batched reciprocal per stack, then row-scatter DMAs +
            # partition_broadcast, one normalize-multiply per head pair.
            pt_t = []
            den_ps = [None, None]
            pv_list = []
            for t in range(NT):
                for j in range(2):
                    h = 2 * t + j
                    po = j * 64
                    s_ps = patt.tile([P, QS], F32, tag="scores")
                    nc.tensor.matmul(
                        s_ps[:], kt_t[t][po:po + 64, :], qt_t[t][po:po + 64, :],
                        start=True, stop=True,
                    )
                    pt = dp.tile([P, QS], BF, tag=f"pt{h}")
                    nc.scalar.activation(
                        pt[:], s_ps[:], AF.Exp, bias=cb_t[h][:], scale=SCALE
                    )
                    pt_t.append(pt)
                    half = h // 8
                    if h % 8 == 0:
                        dps = psml.tile([16, QS], F32, tag=f"den{half}")
                        den_ps[half] = dps
                    nc.tensor.matmul(
                        den_ps[half][:], oh[:, h * 16:h * 16 + 16], pt[:],
                        start=(h % 8 == 0), stop=(h % 8 == 7),
                    )
                pv_ps = patt.tile([P, QS], F32, tag="pv")
                nc.tensor.matmul(
                    pv_ps[0:64, :], v_sb[:, (2 * t) * 64:(2 * t) * 64 + 64],
                    pt_t[2 * t][:], start=True, stop=True, tile_position=(0, 0),
                )
                nc.tensor.matmul(
                    pv_ps[64:128, :], v_sb[:, (2 * t + 1) * 64:(2 * t + 1) * 64 + 64],
                    pt_t[2 * t + 1][:], start=True, stop=True, tile_position=(0, 64),
                )
                pv_list.append(pv_ps)

            rc_half = []
            for half in range(2):
                rc = fp.tile([16, QS], F32, tag=f"rchalf{half}")
                nc.vector.reciprocal_approx_fast(rc[0:8, :], den_ps[half][0:8, :])
                rc_half.append(rc)
            ot_t = []
            for t in range(NT):
                ot = dp.tile([P, QS], BF, tag=f"ot{t}")
                for j in range(2):
                    h = 2 * t + j
                    po = j * 64
                    r0 = fp.tile([1, QS], F32, tag=f"rcp0_{h % 4}")
                    nc.sync.dma_start(
                        r0[:], rc_half[h // 8][(h % 8):(h % 8) + 1, :]
                    )
                    # NB: partition_broadcast with an offset output base silently
                    # writes nothing on HW -- always broadcast to a full tile.
                    rc_bc = fp.tile([P, QS], F32, tag="rcbc")
                    nc.gpsimd.partition_broadcast(rc_bc[:], r0[:], channels=P)
                    nc.vector.tensor_mul(
                        ot[po:po + 64, :], pv_list[t][po:po + 64, :],
                        rc_bc[po:po + 64, :],
                    )
                ot_t.append(ot)

            # ---- output projection out[q, d] = outT^T Wo
            # (stationary ot slice reused across both 512-blocks) ----
            for qi in range(QS // P):
                ops = [pacc.tile([P, 512], F32, tag="acc", name=f"ops{qi}_{_b}") for _b in range(2)]
                for tt in range(NT):
                    for blk in range(2):
                        nc.tensor.matmul(
                            ops[blk][:], ot_t[tt][:, qi * P:(qi + 1) * P],
                            wo_t[tt][:, blk * 512:(blk + 1) * 512],
                            start=(tt == 0), stop=(tt == NT - 1),
                        )
                o_sb = fp.tile([P, 2, 512], F32, tag="osb")
                for blk in range(2):
                    nc.vector.tensor_copy(o_sb[:, blk], ops[blk][:])
                    rings[blk].dma_start(
                        out[qi * P:(qi + 1) * P, blk * 512:(blk + 1) * 512],
                        o_sb[:, blk],
                    )

    nc.compile()
    return nc


def _get_nc():
    if "nc" not in _CACHE:
        _CACHE["nc"] = _build()
    return _CACHE["nc"]


def _in_maps(x, Wq, bq, Wk, bk, Wv, bv, Wo, bo):
    bf = ml_dtypes.bfloat16
    f32 = np.float32
    x = np.asarray(x, f32)
    xT = np.ascontiguousarray(np.transpose(x, (0, 2, 1)))  # [B, D, S]
    wq = np.asarray(Wq, f32).astype(bf)
    wk = np.asarray(Wk, f32).astype(bf)
    wv = np.asarray(Wv, f32).astype(bf)
    wo = np.asarray(Wo, f32).astype(bf)
    bq2 = np.ascontiguousarray(np.asarray(bq, f32).reshape(NT, P).T)
    bk2 = np.ascontiguousarray(np.asarray(bk, f32).reshape(NT, P).T)
    bv2 = np.asarray(bv, f32).astype(bf).reshape(1, D)
    slopes = 1.0 / 2.0 ** (np.arange(H, dtype=np.float64) / H)
    ks = np.arange(K0, S, dtype=np.float64)
    cbias = np.ascontiguousarray(
        (slopes[:, None] * (ks[None, :] - (S - 1))).astype(f32).T
    )
    maps = []
    for c in range(N_CORES):
        b, q0 = c // 4, (c % 4) * QS
        maps.append({
            "xTq": np.ascontiguousarray(xT[b, :, q0:q0 + QS]).astype(bf),
            "xTk": np.ascontiguousarray(xT[b, :, K0:S]).astype(bf),
            "Wq": wq, "Wk": wk, "Wv": wv, "Wo": wo,
            "bq": bq2, "bk": bk2, "bv": bv2, "cbias": cbias,
        })
    return maps


def _run(inputs, trace=False, tmpdir=None):
    from concourse.bass_utils import run_bass_kernel_spmd

    nc = _get_nc()
    maps = _in_maps(**inputs)
    res = run_bass_kernel_spmd(
        nc, maps, core_ids=list(range(N_CORES)), trace=trace, tmpdir=tmpdir
    )
    bo = np.asarray(inputs["bo"], np.float32)
    full = np.zeros((B, S, D), np.float32)
    for c in range(N_CORES):
        b, q0 = c // 4, (c % 4) * QS
        full[b, q0:q0 + QS] = res.results[c]["out"]
    full += bo[None, None, :]
    return full, res


def kernel(**inputs) -> np.ndarray:
    return _run(inputs, trace=False)[0]


# revision 24
# speedup vs baseline: 1.0108x; 1.0108x over previous
"""ALiBi attention (B=2, S=2048, D=1024, H=16) on 8 TRN2 NeuronCores.

Sharding: core c handles batch b = c//4 and query slice qs = (c%4)*512.
Key insight: the reference applies bias slope_h*(k-q) with NO causal mask and
slopes in [0.52, 1.0], so softmax mass sits entirely on the last ~60 keys.
Keeping only the last KW=128 keys gives max attention-weight error ~1e-29.
Furthermore exp(qk*scale + slope*(k-q) - rowmax(q)) with rowmax ~= slope*(S-1-q)
reduces to exp(qk*scale + slope*(k-S+1)): the bias is purely a function of k,
i.e. a per-partition constant in the [k, q] layout -> single fused ACT op.

Per core:
  QT[ch,q]   = Wq^T x^T        (8 ch-tiles x 8 d-tiles, N=512)
  KT[ch,k]   = Wk^T xk^T       (8 x 8, N=128)
  V[k,ch]    = xk Wv           (2 blocks x 8 d-tiles, N=512) + ones col per head
  PT_h[k,q]  = exp(scale*KT_h^T QT_h + cbias_h)      (ACT, per-partition bias)
  denom      = ones^T PT_h     -> reciprocal -> PE outer-product broadcast
  outT_h     = V_h^T PT_h, normalized by denom       (col-tiled into head pairs)
  out[q,d]   = outT^T Wo       (+ bo on host)
No collectives: cores are fully independent; host concatenates query slices.
"""

import numpy as np
import ml_dtypes

D = 1024
H = 16
HD = 64
B = 2
S = 2048
QS = 512          # queries per core
KW = 128          # key window (last KW keys carry all softmax mass)
K0 = S - KW
NT = 8            # 128-wide tiles over D
P = 128
SCALE = HD ** -0.5
N_CORES = 8

_CACHE = {}


def _build():
    import concourse.bacc as bacc
    import concourse.mybir as mybir
    import concourse.tile as tile
    from concourse.masks import make_identity

    BF = mybir.dt.bfloat16
    F32 = mybir.dt.float32
    AF = mybir.ActivationFunctionType

    nc = bacc.Bacc("TRN2", target_bir_lowering=False, debug=False, num_devices=N_CORES)

    xTq = nc.dram_tensor("xTq", [D, QS], BF, kind="ExternalInput").ap()
    xTk = nc.dram_tensor("xTk", [D, KW], BF, kind="ExternalInput").ap()
    Wq = nc.dram_tensor("Wq", [D, D], BF, kind="ExternalInput").ap()
    Wk = nc.dram_tensor("Wk", [D, D], BF, kind="ExternalInput").ap()
    Wv = nc.dram_tensor("Wv", [D, D], BF, kind="ExternalInput").ap()
    Wo = nc.dram_tensor("Wo", [D, D], BF, kind="ExternalInput").ap()
    bq = nc.dram_tensor("bq", [P, NT], F32, kind="ExternalInput").ap()
    bk = nc.dram_tensor("bk", [P, NT], F32, kind="ExternalInput").ap()
    bv = nc.dram_tensor("bv", [1, D], BF, kind="ExternalInput").ap()
    cb = nc.dram_tensor("cbias", [KW, H], F32, kind="ExternalInput").ap()
    out = nc.dram_tensor("out", [QS, D], F32, kind="ExternalOutput").ap()

    with tile.TileContext(nc) as tc:
        with (
            tc.tile_pool(name="wpool", bufs=1) as wp,
            tc.tile_pool(name="dpool", bufs=1) as dp,
            tc.tile_pool(name="flow", bufs=3) as fp,
            tc.tile_pool(name="pacc", bufs=2, space="PSUM") as pacc,
            tc.tile_pool(name="patt", bufs=2, space="PSUM") as patt,
            tc.tile_pool(name="psc", bufs=3, space="PSUM") as psc,
            tc.tile_pool(name="psml", bufs=1, space="PSUM") as psml,
        ):
            # ---- load inputs. Per-d-tile chunks (256KB) alternating across the
            # two HWDGE rings (sync + scalar) so accumulation loops can start on
            # chunk 0 instead of waiting for a whole 2MB tensor. Tensor order =
            # compute need order: xk, wk (K), xq, wq (QT), wv (V), wo (outproj).
            rings = [nc.sync, nc.scalar]

            def load_chunked(name, pool, src, cols, first=0):
                a = pool.tile([P, NT, cols], BF, tag=name, name=name)
                src3 = src.rearrange("(t p) c -> p t c", p=P)
                for t in range(first, NT):
                    rings[t % 2].dma_start(a[:, t], src3[:, t])
                return a

            # QT is the biggest dense PE phase: feed it first (xq + wq), then
            # tiny tensors, then xk/wk (K path), wv, wo.
            xq_a = load_chunked("xq_a", dp, xTq, QS)
            wq_a = wp.tile([P, NT, D], BF, tag="wq_a")
            wq_src = Wq.rearrange("(t p) c -> p t c", p=P)
            for t in range(4):
                rings[t % 2].dma_start(wq_a[:, t], wq_src[:, t])
            bq_a = dp.tile([P, NT], F32, tag="bq_a")
            nc.sync.dma_start(bq_a[:], bq[:])
            bk_a = dp.tile([P, NT], F32, tag="bk_a")
            nc.scalar.dma_start(bk_a[:], bk[:])
            cb_a = dp.tile([KW, H], F32, tag="cb_a")
            nc.sync.dma_start(cb_a[:], cb[:])
            bv_sb = dp.tile([1, D], BF, tag="bvsb")
            nc.scalar.dma_start(bv_sb[:], bv[:])
            bq_t = [bq_a[:, t:t + 1] for t in range(NT)]
            bk_t = [bk_a[:, t:t + 1] for t in range(NT)]
            cb_t = [cb_a[:, h:h + 1] for h in range(H)]
            for t in range(4, NT):
                rings[t % 2].dma_start(wq_a[:, t], wq_src[:, t])

            xk_a = dp.tile([P, NT, KW], BF, tag="xk_a")
            nc.sync.dma_start(xk_a[:], xTk.rearrange("(t p) k -> p t k", p=P))
            wk_a = load_chunked("wk_a", wp, Wk, D)
            wv_a = load_chunked("wv_a", wp, Wv, D)
            wo_a = load_chunked("wo_a", wp, Wo, D)
            xk_t = [xk_a[:, t] for t in range(NT)]
            wk_t = [wk_a[:, t] for t in range(NT)]
            wv_t = [wv_a[:, t] for t in range(NT)]
            xq_t = [xq_a[:, t] for t in range(NT)]
            wq_t = [wq_a[:, t] for t in range(NT)]
            wo_t = [wo_a[:, t] for t in range(NT)]

            ones_row = dp.tile([1, P], BF, tag="ones_row")
            nc.vector.memset(ones_row[:], 1.0)
            # one-hot stationaries: oh[:, h*16 + (h%8)] = 1 -> den-stack matmuls
            oh = dp.tile([P, H * 16], BF, tag="onehot")
            nc.vector.memset(oh[:], 0.0)
            for h in range(H):
                nc.vector.memset(oh[:, h * 16 + (h % 8):h * 16 + (h % 8) + 1], 1.0)

            # ---- QT[ch, q]: d-outer over ch-tile pairs, paced by wq chunk DMAs ----
            qt_t = []
            for pr in range(NT // 2):
                ps2 = [
                    pacc.tile([P, QS], F32, tag="acc", name=f"qps{pr}_{j}")
                    for j in range(2)
                ]
                for d in range(NT):
                    for j in range(2):
                        t = 2 * pr + j
                        nc.tensor.matmul(
                            ps2[j][:], wq_t[d][:, t * P:(t + 1) * P], xq_t[d][:],
                            start=(d == 0), stop=(d == NT - 1),
                        )
                for j in range(2):
                    t = 2 * pr + j
                    qt = dp.tile([P, QS], BF, tag=f"qt{t}", name=f"qt{t}")
                    nc.vector.tensor_scalar_add(qt[:], ps2[j][:], bq_t[t][:])
                    qt_t.append(qt)

            identity = dp.tile([P, P], BF, tag="identity")
            make_identity(nc, identity[:])

            # ---- K[k, ch] = xk^T Wk (16 big matmuls, d-outer, paced by wk
            # chunks), then 8 PE transposes -> KT[ch, k] + bias ----
            k_sb = dp.tile([P, D], BF, tag="ksb")
            kps = [
                pacc.tile([P, 512], F32, tag="acc", name=f"kps{_b}")
                for _b in range(2)
            ]
            for d in range(NT):
                for blk in range(2):
                    nc.tensor.matmul(
                        kps[blk][:], xk_t[d][:],
                        wk_t[d][:, blk * 512:(blk + 1) * 512],
                        start=(d == 0), stop=(d == NT - 1),
                    )
            for blk in range(2):
                nc.vector.tensor_copy(k_sb[:, blk * 512:(blk + 1) * 512], kps[blk][:])
            kt_t = []
            for t in range(NT):
                tps = psc.tile([P, P], BF, tag="scores", name=f"tps{t}")
                nc.tensor.transpose(tps[:], k_sb[:, t * P:(t + 1) * P], identity[:])
                kt = dp.tile([P, KW], BF, tag=f"kt{t}", name=f"kt{t}")
                nc.vector.tensor_scalar_add(kt[:], tps[:], bk_t[t][:])
                kt_t.append(kt)

            # ---- V[k, ch] (stationary xk_t[d] reused across both 512-blocks) ----
            v_sb = dp.tile([P, D], BF, tag="vsb")
            vps = [pacc.tile([P, 512], F32, tag="acc", name=f"vps{_b}") for _b in range(2)]
            for d in range(NT):
                for blk in range(2):
                    nc.tensor.matmul(
                        vps[blk][:], xk_t[d][:],
                        wv_t[d][:, blk * 512:(blk + 1) * 512],
                        start=(d == 0), stop=False,
                    )
            for blk in range(2):
                nc.tensor.matmul(
                    vps[blk][:], ones_row[:], bv_sb[:, blk * 512:(blk + 1) * 512],
                    start=False, stop=True,
                )
                nc.vector.tensor_copy(v_sb[:, blk * 512:(blk + 1) * 512], vps[blk][:])

            # ---- attention ----
            # Phase A per head: QK -> exp(PT) -> one-hot den-stack matmul; PV pairs.
            # Denominators for heads 0-7 accumulate in den_ps[0], 8-15 in den_ps[1]
            # (rows h%8). One batched reciprocal per stack, then row-scatter DMAs +
            # partition_broadcast, one normalize-multiply per head pair.
            pt_t = []
            den_ps = [None, None]
            pv_list = []
            for t in range(NT):
                # QK pair back-to-back: lhsT base partitions 0/64 -> row-tiled,
                # the two matmuls run concurrently in the PE array.
                sps2 = []
                for j in range(2):
                    po = j * 64
                    s_ps = psc.tile([P, QS], F32, tag="scores", name=f"sps{t}_{j}")
                    nc.tensor.matmul(
                        s_ps[:], kt_t[t][po:po + 64, :], qt_t[t][po:po + 64, :],
                        start=True, stop=True,
                    )
                    sps2.append(s_ps)
                for j in range(2):
                    h = 2 * t + j
                    pt = dp.tile([P, QS], BF, tag=f"pt{h}", name=f"pt{h}")
                    nc.scalar.activation(
                        pt[:], sps2[j][:], AF.Exp, bias=cb_t[h][:], scale=SCALE
                    )
                    pt_t.append(pt)
                for j in range(2):
                    h = 2 * t + j
                    half = h // 8
                    if h % 8 == 0:
                        dps = psml.tile([16, QS], F32, tag="den", name=f"den{half}")
                        den_ps[half] = dps
                    nc.tensor.matmul(
                        den_ps[half][:], oh[:, h * 16:h * 16 + 16], pt_t[h][:],
                        start=(h % 8 == 0), stop=(h % 8 == 7),
                    )
                # PV pair back-to-back: col-tiled (0,0)/(0,64), concurrent.
                pv_ps = patt.tile([P, QS], F32, tag="pv")
                nc.tensor.matmul(
                    pv_ps[0:64, :], v_sb[:, (2 * t) * 64:(2 * t) * 64 + 64],
                    pt_t[2 * t][:], start=True, stop=True, tile_position=(0, 0),
                )
                nc.tensor.matmul(
                    pv_ps[64:128, :], v_sb[:, (2 * t + 1) * 64:(2 * t + 1) * 64 + 64],
                    pt_t[2 * t + 1][:], start=True, stop=True, tile_position=(0, 64),
                )
                pv_list.append(pv_ps)

            rc_half = []
            for half in range(2):
                rc = fp.tile([16, QS], F32, tag=f"rchalf{half}")
                nc.vector.reciprocal_approx_fast(rc[0:8, :], den_ps[half][0:8, :])
                rc_half.append(rc)
            ot_t = []
            for t in range(NT):
                ot = dp.tile([P, QS], BF, tag=f"ot{t}")
                for j in range(2):
                    h = 2 * t + j
                    po = j * 64
                    r0 = fp.tile([1, QS], F32, tag=f"rcp0_{h % 4}")
                    nc.sync.dma_start(
                        r0[:], rc_half[h // 8][(h % 8):(h % 8) + 1, :]
                    )
                    # NB: partition_broadcast with an offset output base silently
                    # writes nothing on HW -- always broadcast to a full tile.
                    rc_bc = fp.tile([P, QS], F32, tag="rcbc")
                    nc.gpsimd.partition_broadcast(rc_bc[:], r0[:], channels=P)
                    nc.vector.tensor_mul(
                        ot[po:po + 64, :], pv_list[t][po:po + 64, :],
                        rc_bc[po:po + 64, :],
                    )
                ot_t.append(ot)

            # ---- output projection out[q, d] = outT^T Wo
            # (stationary ot slice reused across both 512-blocks) ----
            for qi in range(QS // P):
                ops = [pacc.tile([P, 512], F32, tag="acc", name=f"ops{qi}_{_b}") for _b in range(2)]
                for tt in range(NT):
                    for blk in range(2):
                        nc.tensor.matmul(
                            ops[blk][:], ot_t[tt][:, qi * P:(qi + 1) * P],
                            wo_t[tt][:, blk * 512:(blk + 1) * 512],
                            start=(tt == 0), stop=(tt == NT - 1),
                        )
                o_sb = fp.tile([P, 2, 512], F32, tag="osb")
                for blk in range(2):
                    nc.vector.tensor_copy(o_sb[:, blk], ops[blk][:])
                    rings[blk].dma_start(
                        out[qi * P:(qi + 1) * P, blk * 512:(blk + 1) * 512],
                        o_sb[:, blk],
                    )

    nc.compile()
    return nc


def _get_nc():
    if "nc" not in _CACHE:
        _CACHE["nc"] = _build()
    return _CACHE["nc"]


def _in_maps(x, Wq, bq, Wk, bk, Wv, bv, Wo, bo):
    bf = ml_dtypes.bfloat16
    f32 = np.float32
    x = np.asarray(x, f32)
    xT = np.ascontiguousarray(np.transpose(x, (0, 2, 1)))  # [B, D, S]
    wq = np.asarray(Wq, f32).astype(bf)
    wk = np.asarray(Wk, f32).astype(bf)
    wv = np.asarray(Wv, f32).astype(bf)
    wo = np.asarray(Wo, f32).astype(bf)
    bq2 = np.ascontiguousarray(np.asarray(bq, f32).reshape(NT, P).T)
    bk2 = np.ascontiguousarray(np.asarray(bk, f32).reshape(NT, P).T)
    bv2 = np.asarray(bv, f32).astype(bf).reshape(1, D)
    slopes = 1.0 / 2.0 ** (np.arange(H, dtype=np.float64) / H)
    ks = np.arange(K0, S, dtype=np.float64)
    cbias = np.ascontiguousarray(
        (slopes[:, None] * (ks[None, :] - (S - 1))).astype(f32).T
    )
    maps = []
    for c in range(N_CORES):
        b, q0 = c // 4, (c % 4) * QS
        maps.append({
            "xTq": np.ascontiguousarray(xT[b, :, q0:q0 + QS]).astype(bf),
            "xTk": np.ascontiguousarray(xT[b, :, K0:S]).astype(bf),
            "Wq": wq, "Wk": wk, "Wv": wv, "Wo": wo,
            "bq": bq2, "bk": bk2, "bv": bv2, "cbias": cbias,
        })
    return maps


def _run(inputs, trace=False, tmpdir=None):
    from concourse.bass_utils import run_bass_kernel_spmd

    nc = _get_nc()
    maps = _in_maps(**inputs)
    res = run_bass_kernel_spmd(
        nc, maps, core_ids=list(range(N_CORES)), trace=trace, tmpdir=tmpdir
    )
    bo = np.asarray(inputs["bo"], np.float32)
    full = np.zeros((B, S, D), np.float32)
    for c in range(N_CORES):
        b, q0 = c // 4, (c % 4) * QS
        full[b, q0:q0 + QS] = res.results[c]["out"]
    full += bo[None, None, :]
    return full, res


def kernel(**inputs) -> np.ndarray:
    return _run(inputs, trace=False)[0]


# revision 26
# speedup vs baseline: 1.0482x; 1.0369x over previous
"""ALiBi attention (B=2, S=2048, D=1024, H=16) on 8 TRN2 NeuronCores.

Sharding: core c handles batch b = c//4 and query slice qs = (c%4)*512.
Key insight: the reference applies bias slope_h*(k-q) with NO causal mask and
slopes in [0.52, 1.0], so softmax mass sits entirely on the last ~60 keys.
Keeping only the last KW=128 keys gives max attention-weight error ~1e-29.
Furthermore exp(qk*scale + slope*(k-q) - rowmax(q)) with rowmax ~= slope*(S-1-q)
reduces to exp(qk*scale + slope*(k-S+1)): the bias is purely a function of k,
i.e. a per-partition constant in the [k, q] layout -> single fused ACT op.

Per core:
  QT[ch,q]   = Wq^T x^T        (8 ch-tiles x 8 d-tiles, N=512)
  KT[ch,k]   = Wk^T xk^T       (8 x 8, N=128)
  V[k,ch]    = xk Wv           (2 blocks x 8 d-tiles, N=512) + ones col per head
  PT_h[k,q]  = exp(scale*KT_h^T QT_h + cbias_h)      (ACT, per-partition bias)
  denom      = ones^T PT_h     -> reciprocal -> PE outer-product broadcast
  outT_h     = V_h^T PT_h, normalized by denom       (col-tiled into head pairs)
  out[q,d]   = outT^T Wo       (+ bo on host)
No collectives: cores are fully independent; host concatenates query slices.
"""

import numpy as np
import ml_dtypes

D = 1024
H = 16
HD = 64
B = 2
S = 2048
QS = 512          # queries per core
KW = 128          # key window (last KW keys carry all softmax mass)
K0 = S - KW
NT = 8            # 128-wide tiles over D
P = 128
SCALE = HD ** -0.5
N_CORES = 8

_CACHE = {}


def _build():
    import concourse.bacc as bacc
    import concourse.mybir as mybir
    import concourse.tile as tile
    from concourse.masks import make_identity

    BF = mybir.dt.bfloat16
    F32 = mybir.dt.float32
    AF = mybir.ActivationFunctionType

    nc = bacc.Bacc("TRN2", target_bir_lowering=False, debug=False, num_devices=N_CORES)

    xTq = nc.dram_tensor("xTq", [D, QS], BF, kind="ExternalInput").ap()
    xTk = nc.dram_tensor("xTk", [D, KW], BF, kind="ExternalInput").ap()
    Wq = nc.dram_tensor("Wq", [D, D], BF, kind="ExternalInput").ap()
    Wk = nc.dram_tensor("Wk", [D, D], BF, kind="ExternalInput").ap()
    Wv = nc.dram_tensor("Wv", [D, D], BF, kind="ExternalInput").ap()
    Wo = nc.dram_tensor("Wo", [D, D], BF, kind="ExternalInput").ap()
    bq = nc.dram_tensor("bq", [P, NT], F32, kind="ExternalInput").ap()
    bk = nc.dram_tensor("bk", [P, NT], F32, kind="ExternalInput").ap()
    bv = nc.dram_tensor("bv", [1, D], BF, kind="ExternalInput").ap()
    cb = nc.dram_tensor("cbias", [KW, H], F32, kind="ExternalInput").ap()
    out = nc.dram_tensor("out", [QS, D], F32, kind="ExternalOutput").ap()

    with tile.TileContext(nc) as tc:
        with (
            tc.tile_pool(name="wpool", bufs=1) as wp,
            tc.tile_pool(name="dpool", bufs=1) as dp,
            tc.tile_pool(name="flow", bufs=3) as fp,
            tc.tile_pool(name="pacc", bufs=2, space="PSUM") as pacc,
            tc.tile_pool(name="patt", bufs=2, space="PSUM") as patt,
            tc.tile_pool(name="psc", bufs=3, space="PSUM") as psc,
            tc.tile_pool(name="psml", bufs=1, space="PSUM") as psml,
        ):
            # ---- load inputs. Per-d-tile chunks (256KB) alternating across the
            # two HWDGE rings (sync + scalar) so accumulation loops can start on
            # chunk 0 instead of waiting for a whole 2MB tensor. Tensor order =
            # compute need order: xk, wk (K), xq, wq (QT), wv (V), wo (outproj).
            rings = [nc.sync, nc.scalar]

            def load_chunked(name, pool, src, cols, first=0):
                a = pool.tile([P, NT, cols], BF, tag=name, name=name)
                src3 = src.rearrange("(t p) c -> p t c", p=P)
                for t in range(first, NT):
                    rings[t % 2].dma_start(a[:, t], src3[:, t])
                return a

            # QT is the biggest dense PE phase: feed it first (xq + wq), then
            # tiny tensors, then xk/wk (K path), wv, wo.
            xq_a = load_chunked("xq_a", dp, xTq, QS)
            wq_a = wp.tile([P, NT, D], BF, tag="wq_a")
            wq_src = Wq.rearrange("(t p) c -> p t c", p=P)
            for t in range(4):
                rings[t % 2].dma_start(wq_a[:, t], wq_src[:, t])
            bq_a = dp.tile([P, NT], F32, tag="bq_a")
            nc.sync.dma_start(bq_a[:], bq[:])
            bk_a = dp.tile([P, NT], F32, tag="bk_a")
            nc.scalar.dma_start(bk_a[:], bk[:])
            cb_a = dp.tile([KW, H], F32, tag="cb_a")
            nc.sync.dma_start(cb_a[:], cb[:])
            bv_sb = dp.tile([1, D], BF, tag="bvsb")
            nc.scalar.dma_start(bv_sb[:], bv[:])
            bq_t = [bq_a[:, t:t + 1] for t in range(NT)]
            bk_t = [bk_a[:, t:t + 1] for t in range(NT)]
            cb_t = [cb_a[:, h:h + 1] for h in range(H)]
            for t in range(4, NT):
                rings[t % 2].dma_start(wq_a[:, t], wq_src[:, t])

            xk_a = dp.tile([P, NT, KW], BF, tag="xk_a")
            nc.sync.dma_start(xk_a[:], xTk.rearrange("(t p) k -> p t k", p=P))
            wk_a = load_chunked("wk_a", wp, Wk, D)
            wv_a = load_chunked("wv_a", wp, Wv, D)
            wo_a = load_chunked("wo_a", wp, Wo, D)
            xk_t = [xk_a[:, t] for t in range(NT)]
            wk_t = [wk_a[:, t] for t in range(NT)]
            wv_t = [wv_a[:, t] for t in range(NT)]
            xq_t = [xq_a[:, t] for t in range(NT)]
            wq_t = [wq_a[:, t] for t in range(NT)]
            wo_t = [wo_a[:, t] for t in range(NT)]

            ones_row = dp.tile([1, P], BF, tag="ones_row")
            nc.vector.memset(ones_row[:], 1.0)
            ones_col = dp.tile([P, 1], BF, tag="ones_col")
            nc.vector.memset(ones_col[:], 1.0)

            # ---- QT[ch, q]: d-outer over ch-tile pairs, paced by wq chunk DMAs ----
            qt_t = []
            for pr in range(NT // 2):
                ps2 = [
                    pacc.tile([P, QS], F32, tag="acc", name=f"qps{pr}_{j}")
                    for j in range(2)
                ]
                for d in range(NT):
                    for j in range(2):
                        t = 2 * pr + j
                        nc.tensor.matmul(
                            ps2[j][:], wq_t[d][:, t * P:(t + 1) * P], xq_t[d][:],
                            start=(d == 0), stop=(d == NT - 1),
                        )
                for j in range(2):
                    t = 2 * pr + j
                    qt = dp.tile([P, QS], BF, tag=f"qt{t}", name=f"qt{t}")
                    nc.vector.tensor_scalar_add(qt[:], ps2[j][:], bq_t[t][:])
                    qt_t.append(qt)

            identity = dp.tile([P, P], BF, tag="identity")
            make_identity(nc, identity[:])

            # ---- K[k, ch] = xk^T Wk (16 big matmuls, d-outer, paced by wk
            # chunks), then 8 PE transposes -> KT[ch, k] + bias ----
            k_sb = dp.tile([P, D], BF, tag="ksb")
            kps = [
                pacc.tile([P, 512], F32, tag="acc", name=f"kps{_b}")
                for _b in range(2)
            ]
            for d in range(NT):
                for blk in range(2):
                    nc.tensor.matmul(
                        kps[blk][:], xk_t[d][:],
                        wk_t[d][:, blk * 512:(blk + 1) * 512],
                        start=(d == 0), stop=(d == NT - 1),
                    )
            for blk in range(2):
                nc.vector.tensor_copy(k_sb[:, blk * 512:(blk + 1) * 512], kps[blk][:])
            kt_t = []
            for t in range(NT):
                tps = psc.tile([P, P], BF, tag="scores", name=f"tps{t}")
                nc.tensor.transpose(tps[:], k_sb[:, t * P:(t + 1) * P], identity[:])
                kt = dp.tile([P, KW], BF, tag=f"kt{t}", name=f"kt{t}")
                nc.vector.tensor_scalar_add(kt[:], tps[:], bk_t[t][:])
                kt_t.append(kt)

            # ---- V[k, ch] (stationary xk_t[d] reused across both 512-blocks) ----
            v_sb = dp.tile([P, D], BF, tag="vsb")
            vps = [pacc.tile([P, 512], F32, tag="acc", name=f"vps{_b}") for _b in range(2)]
            for d in range(NT):
                for blk in range(2):
                    nc.tensor.matmul(
                        vps[blk][:], xk_t[d][:],
                        wv_t[d][:, blk * 512:(blk + 1) * 512],
                        start=(d == 0), stop=False,
                    )
            for blk in range(2):
                nc.tensor.matmul(
                    vps[blk][:], ones_row[:], bv_sb[:, blk * 512:(blk + 1) * 512],
                    start=False, stop=True,
                )
                nc.vector.tensor_copy(v_sb[:, blk * 512:(blk + 1) * 512], vps[blk][:])

            # ---- attention ----
            # Phase A per head: QK -> exp(PT) -> one-hot den-stack matmul; PV pairs.
            # Denominators for heads 0-7 accumulate in den_ps[0], 8-15 in den_ps[1]
            # (rows h%8). One batched reciprocal per stack, then row-scatter DMAs +
            # partition_broadcast, one normalize-multiply per head pair.
            # Per head: QK (row-tiled pairs) -> exp -> denom matmul -> fast
            # reciprocal (fp32, [1,512] psum at p0) -> partition_broadcast
            # (gpsimd DMA; NB offset output base silently writes nothing on HW,
            # so broadcast to a full 128-partition tile) -> normalize-multiply.
            # Each head's chain completes ~3us after its exp, so ot tiles arrive
            # progressively and the out-projection never stalls on the last head.
            pt_t = []
            ot_t = []
            for t in range(NT):
                sps2 = []
                for j in range(2):
                    po = j * 64
                    s_ps = psc.tile([P, QS], F32, tag="scores", name=f"sps{t}_{j}")
                    nc.tensor.matmul(
                        s_ps[:], kt_t[t][po:po + 64, :], qt_t[t][po:po + 64, :],
                        start=True, stop=True,
                    )
                    sps2.append(s_ps)
                for j in range(2):
                    h = 2 * t + j
                    pt = dp.tile([P, QS], BF, tag=f"pt{h % 4}", name=f"pt{h}")
                    nc.scalar.activation(
                        pt[:], sps2[j][:], AF.Exp, bias=cb_t[h][:], scale=SCALE
                    )
                    pt_t.append(pt)
                # PV pair back-to-back: col-tiled (0,0)/(0,64), concurrent.
                pv_ps = patt.tile([P, QS], F32, tag="pv")
                nc.tensor.matmul(
                    pv_ps[0:64, :], v_sb[:, (2 * t) * 64:(2 * t) * 64 + 64],
                    pt_t[2 * t][:], start=True, stop=True, tile_position=(0, 0),
                )
                nc.tensor.matmul(
                    pv_ps[64:128, :], v_sb[:, (2 * t + 1) * 64:(2 * t + 1) * 64 + 64],
                    pt_t[2 * t + 1][:], start=True, stop=True, tile_position=(0, 64),
                )
                ot = dp.tile([P, QS], BF, tag=f"ot{t}", name=f"ot{t}")
                for j in range(2):
                    h = 2 * t + j
                    po = j * 64
                    d_ps = psml.tile([1, QS], F32, tag="den", name=f"dps{h}")
                    nc.tensor.matmul(
                        d_ps[:], ones_col[:], pt_t[h][:], start=True, stop=True
                    )
                    r0 = fp.tile([1, QS], F32, tag=f"rcp0_{h % 4}", name=f"r0_{h}")
                    nc.vector.reciprocal_approx_fast(r0[:], d_ps[:])
                    rc_bc = fp.tile([P, QS], F32, tag="rcbc", name=f"rcbc{h}")
                    nc.gpsimd.partition_broadcast(rc_bc[:], r0[:], channels=P)
                    nc.vector.tensor_mul(
                        ot[po:po + 64, :], pv_ps[po:po + 64, :],
                        rc_bc[po:po + 64, :],
                    )
                ot_t.append(ot)

            # ---- output projection out[q, d] = outT^T Wo
            # (stationary ot slice reused across both 512-blocks) ----
            for qi in range(QS // P):
                ops = [pacc.tile([P, 512], F32, tag="acc", name=f"ops{qi}_{_b}") for _b in range(2)]
                for tt in range(NT):
                    for blk in range(2):
                        nc.tensor.matmul(
                            ops[blk][:], ot_t[tt][:, qi * P:(qi + 1) * P],
                            wo_t[tt][:, blk * 512:(blk + 1) * 512],
                            start=(tt == 0), stop=(tt == NT - 1),
                        )
                o_sb = fp.tile([P, 2, 512], F32, tag="osb")
                for blk in range(2):
                    nc.vector.tensor_copy(o_sb[:, blk], ops[blk][:])
                    rings[blk].dma_start(
                        out[qi * P:(qi + 1) * P, blk * 512:(blk + 1) * 512],
                        o_sb[:, blk],
                    )

    nc.compile()
    return nc


def _get_nc():
    if "nc" not in _CACHE:
        _CACHE["nc"] = _build()
    return _CACHE["nc"]


def _in_maps(x, Wq, bq, Wk, bk, Wv, bv, Wo, bo):
    bf = ml_dtypes.bfloat16
    f32 = np.float32
    x = np.asarray(x, f32)
    xT = np.ascontiguousarray(np.transpose(x, (0, 2, 1)))  # [B, D, S]
    wq = np.asarray(Wq, f32).astype(bf)
    wk = np.asarray(Wk, f32).astype(bf)
    wv = np.asarray(Wv, f32).astype(bf)
    wo = np.asarray(Wo, f32).astype(bf)
    bq2 = np.ascontiguousarray(np.asarray(bq, f32).reshape(NT, P).T)
    bk2 = np.ascontiguousarray(np.asarray(bk, f32).reshape(NT, P).T)
    bv2 = np.asarray(bv, f32).astype(bf).reshape(1, D)
    slopes = 1.0 / 2.0 ** (np.arange(H, dtype=np.float64) / H)
    ks = np.arange(K0, S, dtype=np.float64)
    cbias = np.ascontiguousarray(
        (slopes[:, None] * (ks[None, :] - (S - 1))).astype(f32).T
    )
    maps = []
    for c in range(N_CORES):
        b, q0 = c // 4, (c % 4) * QS
        maps.append({
            "xTq": np.ascontiguousarray(xT[b, :, q0:q0 + QS]).astype(bf),
            "xTk": np.ascontiguousarray(xT[b, :, K0:S]).astype(bf),
            "Wq": wq, "Wk": wk, "Wv": wv, "Wo": wo,
            "bq": bq2, "bk": bk2, "bv": bv2, "cbias": cbias,
        })
    return maps


def _run(inputs, trace=False, tmpdir=None):
    from concourse.bass_utils import run_bass_kernel_spmd

    nc = _get_nc()
    maps = _in_maps(**inputs)
    res = run_bass_kernel_spmd(
        nc, maps, core_ids=list(range(N_CORES)), trace=trace, tmpdir=tmpdir
    )
    bo = np.asarray(inputs["bo"], np.float32)
    full = np.zeros((B, S, D), np.float32)
    for c in range(N_CORES):
        b, q0 = c // 4, (c % 4) * QS
        full[b, q0:q0 + QS] = res.results[c]["out"]
    full += bo[None, None, :]
    return full, res


def kernel(**inputs) -> np.ndarray:
    return _run(inputs, trace=False)[0]
